# revision 25
# baseline (speedup 1.0000x reference)
"""Trainium2 Bass kernel for nn_Attention_9594956939856.

Single-head spatial self-attention over 64x64 feature maps:
    q = Wq@x, k = Wk@x, v = Wv@x  (1x1 convs over channels)
    out = gamma * softmax(q^T k) @ v + x

Sharding: data-parallel over batch - 8 samples onto 8 NeuronCores, each core
computes one full sample (C=256, N=4096 tokens, dk=32). No collectives.

Per-core design (all PE matmuls fp8 DoubleRow, 0.5 cyc/col):
  - scores computed transposed s'[j,i] with k j-tiles stationary. q/k are
    projected once with 4 replicas along partitions (weights pre-scaled by
    sqrt(A/8) so the 4x2 replica contraction yields A*score, A = 8*log2(e));
    the DR o-pair reads the same q/k rows twice via stride-0 APs.
  - exp is split across two engines: ACT runs true exp (scale=1/A), DVE runs
    a Schraudolph-style bit-trick: round(A*s + B) saturating-uint8 IS the
    fp8e4m3 bit pattern of ~exp(s) (max rel err ~7%, same order as the fp8
    quantization ACT's own output suffers).
  - attention-weighted sum: vT (built by the v-projection with x as the
    stationary side) and an all-ones lhsT accumulate po0/po1/denominator in
    PSUM; the ones matmul has M=128 so the denominator lands broadcast on
    all 128 partitions.
  - finals on DVE in bf16: y = (po * gamma) * recip(pd) + x_bf.  v-bias is
    folded host-side into the residual (softmax rows sum to 1, so
    out = attn@(v+bv) + .. == attn@v + bv), q/k biases into the projection
    bias, gamma*bv into x_bf. Output is bf16, cast to fp32 on host.
"""

import math

import ml_dtypes
import numpy as np

import concourse.bass as bass
import concourse.mybir as mybir
from concourse.tile import TileContext
from concourse.bass_utils import run_bass_kernel_spmd

B, C, H, W = 8, 256, 64, 64
N = H * W          # 4096 tokens
DK = C // 8        # 32
P = 128
F32 = mybir.dt.float32
BF16 = mybir.dt.bfloat16
FP8 = mybir.dt.float8e4   # IEEE e4m3: bytes >= 120 are inf/nan, max 240
U8 = mybir.dt.uint8
DR = mybir.MatmulPerfMode.DoubleRow
DP = mybir.MatmulPerfMode.DoublePixel
AF = mybir.ActivationFunctionType
ALU = mybir.AluOpType

A_EXP = 8.0 / math.log(2.0)      # 11.5416 - fp8 bits per e-fold
B_SCH = 55.62                    # calibrated for round-to-nearest u8 convert
W_SCALE = math.sqrt(A_EXP / 4.0)  # per-side q/k scale; 4 replicas (DP K=128)

HCH = 512          # i-chunk width
NCH = N // HCH     # 8
NJP = 16           # j-pairs per chunk (32 j-tiles)

# Per-chunk j-pair exp-engine assignment: 7 pairs on DVE (Schraudolph),
# 9 on ACT (true exp); interleaved so both engines stream continuously.
# (PSUM is invisible to both GPSIMD and DMA, so only ACT/DVE can read
# scores - a third exp lane is structurally impossible.)
DVE_JP = frozenset((1, 3, 5, 7, 9, 11, 13))
ATTNV_LAG = 2  # attnv for pair jp emitted after scores of pair jp+LAG

# Diagnostic build modes (timing-only, numerics may be wrong):
#   "pe_free": attnv consumes a constant tile instead of e8 (PE unleashed)
#   "no_attnv": skip attnv+finals (scores+exp floor)
DIAG = {"mode": None}


# ---------------------------------------------------------------------------
# Workaround: the walrus build in this container allows only ONE sync wait
# per instruction ("Too many sync wait commands"), but Tile's wait
# assignment attaches up to 2 (and the tail drain more). Hoist all-but-one
# wait of any over-subscribed instruction onto dedicated same-engine nofuse
# nops inserted immediately before it in the ordered stream.
_PATCHED = False


def _apply_tile_patch():
    global _PATCHED
    if _PATCHED:
        return
    from concourse.tile import TileContext as TC
    from concourse.vector_clock import ScopedClock, VectorClock

    def _drain_and_barrier_split(self, tick_clock, wait_clock):
        gc = tick_clock.global_clock
        n = len(gc)
        for i in range(n):
            if gc[i] > 0:
                vec = [0] * n
                vec[i] = gc[i]
                ins = self.nc.sync.nop(nofuse=True, hint="tail_drain_wait")
                wait_clock.add_sem_waits(
                    ins.ins, ScopedClock({None: VectorClock(vec)})
                )
        self.nc.sync.drain()
        self.nc.all_engine_barrier()
        assert self.sems is not None
        popped = self.nc._tile_sem_poison_stack.pop()
        assert popped is self._sem_poison
        self.nc.clear_and_free_semaphores(list(self.sems.allocated().values()))
        self.nc.all_engine_barrier()

    TC._drain_and_barrier = _drain_and_barrier_split

    orig_lower = TC._lower_ordered_insts
    counter = [0]

    def _lower_split_waits(self, ordered):
        for bb_name, insts in ordered.items():
            new = []
            changed = False
            for inst in insts:
                si = inst.sync_info
                if si is not None and len(si.on_wait) > 1:
                    changed = True
                    waits = list(si.on_wait)
                    for w in waits[:-1]:
                        counter[0] += 1
                        new.append(
                            mybir.InstNoOp(
                                name=f"splitw-{counter[0]}",
                                sync_info=mybir.SyncInfo(
                                    on_wait=[w], on_update=[]
                                ),
                                bass_nofuse=True,
                                engine=inst.engine,
                            )
                        )
                    inst.sync_info = mybir.SyncInfo(
                        on_wait=[waits[-1]], on_update=list(si.on_update)
                    )
                new.append(inst)
            if changed:
                insts[:] = new
        return orig_lower(self, ordered)

    TC._lower_ordered_insts = _lower_split_waits
    _PATCHED = True


def _bcast_o(ap, o=2):
    """Add a stride-0 o-dim of size `o` after the partition dim."""
    p, n = ap.shape
    return ap.rearrange("p (o n) -> p o n", o=1).broadcast_to([p, o, n])


def _emit_body(nc, tc, pools, ext):
    consts, big, epool, fin, ps_big, ps_acc = pools
    x8_e, xb_e, wq8_e, wk8_e, wv8_e, bq_e, bk_e, gam_e, y_e = ext

    # ---- constants / weights ---------------------------------------------
    wq8 = consts.tile([P, 2 * P], FP8, tag="wq8")
    wk8 = consts.tile([P, 2 * P], FP8, tag="wk8")
    wv8 = consts.tile([P, 2 * C], FP8, tag="wv8")
    bq_t = consts.tile([P, 1], F32, tag="bq_t")
    bk_t = consts.tile([P, 1], F32, tag="bk_t")
    gam_t = consts.tile([P, 1], F32, tag="gam_t")
    ones8 = consts.tile([P, 2 * P], FP8, tag="ones8")

    nc.sync.dma_start(out=wq8[:], in_=wq8_e[:])
    nc.sync.dma_start(out=wk8[:], in_=wk8_e[:])
    nc.sync.dma_start(out=wv8[:], in_=wv8_e[:])
    nc.sync.dma_start(out=bq_t[:], in_=bq_e[:])
    nc.sync.dma_start(out=bk_t[:], in_=bk_e[:])
    nc.sync.dma_start(out=gam_t[:], in_=gam_e[:])
    nc.vector.memset(ones8[:], 1.0)

    x8 = big.tile([P, 2 * N], FP8, tag="x8", bufs=2)
    xb = big.tile([P, 2 * N], BF16, tag="xb", bufs=2)
    q8 = big.tile([P, N], FP8, tag="q8", bufs=2)
    k8 = big.tile([P, N], FP8, tag="k8", bufs=2)
    vt8 = big.tile([P, 2 * N], FP8, tag="vt8", bufs=2)

    for h in range(2):
        nc.sync.dma_start(out=x8[:, h * N:(h + 1) * N], in_=x8_e[:, h * N:(h + 1) * N])
    for h in range(2):
        nc.sync.dma_start(out=xb[:, h * N:(h + 1) * N], in_=xb_e[:, h * N:(h + 1) * N])

    x8r = x8[:].rearrange("p (o i) -> p o i", o=2)
    wq8r = wq8[:].rearrange("p (o m) -> p o m", o=2)
    wk8r = wk8[:].rearrange("p (o m) -> p o m", o=2)
    wv8r = wv8[:].rearrange("p (o c) -> p o c", o=2)
    ones8r = ones8[:].rearrange("p (o m) -> p o m", o=2)

    PSB = DIAG.get("ps_bufs", 5)

    def exp_to(e_sl, ps_sl, on_act):
        if DIAG["mode"] == "no_exp":
            nc.vector.memset(e_sl[:, 0:1], 1.0)
        elif (on_act or DIAG["mode"] == "all_act") and DIAG["mode"] != "all_dve":
            nc.scalar.activation(e_sl, ps_sl, AF.Exp, scale=1.0 / A_EXP)
        else:
            nc.vector.tensor_scalar_add(e_sl.bitcast(U8), ps_sl, B_SCH)

    # ---- projections ------------------------------------------------------
    # k, q: contraction over 256 channels = (p, o) via DR; output = 4
    # replicas x 32 dims of scaled q/k; ACT/DVE add bias and cast to fp8.
    for wr, bias_t, dst in ((wk8r, bk_t, k8), (wq8r, bq_t, q8)):
        for c in range(8):
            sl = slice(c * 512, (c + 1) * 512)
            pk = ps_big.tile([P, HCH], F32, tag="ps", bufs=PSB)
            nc.tensor.matmul(pk[:], wr, x8r[:, :, sl], start=True, stop=True,
                             perf_mode=DR)
            if c % 2 == 0:
                nc.scalar.activation(dst[:, sl], pk[:], AF.Identity,
                                     bias=bias_t[:])
            else:
                nc.vector.tensor_scalar_add(dst[:, sl], pk[:], bias_t[:])

    # v: x j-slices stationary, wv8 moving; vt8 layout [h][jp][o][c]
    for t in range(16):
        pv = ps_big.tile([P, HCH], F32, tag="ps", bufs=PSB)
        for o in range(2):
            jt = 2 * t + o
            nc.tensor.matmul(
                pv[:, o * 256:(o + 1) * 256],
                x8r[:, :, jt * P:(jt + 1) * P], wv8r,
                start=True, stop=True, perf_mode=DR,
            )
        pv4 = pv[:].rearrange("p (o h c) -> p o h c", o=2, h=2, c=128)
        for h in range(2):
            o_sl = vt8[:, h * N + t * 256: h * N + (t + 1) * 256]
            out_r = o_sl.rearrange("p (o c) -> p o c", o=2, c=128)
            in_r = pv4[:, :, h, :]
            if (t + h) % 2 == 0:
                nc.scalar.activation(out_r, in_r, AF.Copy)
            else:
                nc.vector.tensor_copy(out_r, in_r)

    # ---- attention main loop ---------------------------------------------
    # j-tile-granular PSUM singles (5-deep rotation); each j-tile's exp goes
    # to ACT or DVE by parity, so a score pair is released in ~one half-exp
    # latency; attnv emission lags scores so the in-order PE queue always
    # has ready work while exps are in flight.
    for ich in range(NCH):
        isl = slice(ich * HCH, (ich + 1) * HCH)
        if DIAG["mode"] not in ("no_exp", "no_attnv"):
            po0 = ps_acc.tile([P, HCH], F32, tag="po0", bufs=1)
            po1 = ps_acc.tile([P, HCH], F32, tag="po1", bufs=1)
            pd = ps_acc.tile([P, HCH], F32, tag="pd", bufs=1)
        else:
            po0 = po1 = pd = None
        rhs_q = q8[:, isl]
        e8_of = {}
        n_av = 0

        def emit_attnv(jp):
            nonlocal n_av
            if DIAG["mode"] == "pe_free":
                e8r = x8[:, 0:1024].rearrange("p (o i) -> p o i", o=2)
            else:
                e8r = e8_of[jp][:].rearrange("p (o i) -> p o i", o=2)
            st, sp = n_av == 0, n_av == NJP - 1
            for h, po in ((0, po0), (1, po1)):
                lhs_v = vt8[:, h * N + jp * 256: h * N + (jp + 1) * 256]
                nc.tensor.matmul(
                    po[:], lhs_v.rearrange("p (o c) -> p o c", o=2), e8r,
                    start=st, stop=sp, perf_mode=DR,
                )
            nc.tensor.matmul(pd[:], ones8r, e8r, start=st, stop=sp,
                             perf_mode=DR)
            n_av += 1

        n_em = 0
        for jt in range(2 * NJP):
            jp, o = jt // 2, jt % 2
            ps = ps_big.tile([P, HCH], F32, tag="ps", bufs=PSB)
            nc.tensor.matmul(ps[:], k8[:, jt * P:(jt + 1) * P], rhs_q,
                             start=True, stop=True, perf_mode=DP)
            if o == 0:
                e8_of[jp] = epool.tile([P, 1024], FP8, tag="e", bufs=20,
                                       name=f"e8_{ich}_{jp}")
            exp_to(e8_of[jp][:, o * HCH:(o + 1) * HCH], ps[:],
                   on_act=(jt % 2 == 0))
            if DIAG["mode"] in ("no_attnv", "no_exp"):
                continue
            while n_em < NJP and 2 * (n_em + ATTNV_LAG) + 1 <= jt:
                emit_attnv(n_em)
                n_em += 1
        if DIAG["mode"] in ("no_attnv", "no_exp"):
            continue
        for jp in range(n_em, NJP):
            emit_attnv(jp)

        dr_bf = fin.tile([P, HCH], BF16, tag="dr", bufs=2)
        with nc.allow_low_precision(reason="bf16 softmax denom; 2e-2 gate"):
            nc.vector.reciprocal(dr_bf[:], pd[:])
        for h, po in ((0, po0), (1, po1)):
            t_bf = fin.tile([P, HCH], BF16, tag=f"t{h}", bufs=2)
            nc.vector.scalar_tensor_tensor(
                t_bf[:], po[:], gam_t[:], dr_bf[:],
                op0=ALU.mult, op1=ALU.mult,
            )
            y_bf = fin.tile([P, HCH], BF16, tag=f"y{h}", bufs=2)
            nc.vector.tensor_tensor(
                y_bf[:], t_bf[:], xb[:, h * N + ich * HCH: h * N + (ich + 1) * HCH],
                op=ALU.add,
            )
            nc.sync.dma_start(
                out=y_e[:, h * N + ich * HCH: h * N + (ich + 1) * HCH],
                in_=y_bf[:],
            )


def build_bass(loop_n: int | None = None) -> bass.Bass:
    """Build the kernel. loop_n wraps the body in a device-side For_i loop
    (with a tiny 'tick' sentinel output) for slope-based benchmarking."""
    _apply_tile_patch()
    nc = bass.Bass()

    x8_e = nc.declare_dram_parameter("x8", [P, 2 * N], FP8, isOutput=False)
    xb_e = nc.declare_dram_parameter("xb", [P, 2 * N], BF16, isOutput=False)
    wq8_e = nc.declare_dram_parameter("wq8", [P, 2 * P], FP8, isOutput=False)
    wk8_e = nc.declare_dram_parameter("wk8", [P, 2 * P], FP8, isOutput=False)
    wv8_e = nc.declare_dram_parameter("wv8", [P, 2 * C], FP8, isOutput=False)
    bq_e = nc.declare_dram_parameter("bq_r", [P, 1], F32, isOutput=False)
    bk_e = nc.declare_dram_parameter("bk_r", [P, 1], F32, isOutput=False)
    gam_e = nc.declare_dram_parameter("gam_b", [P, 1], F32, isOutput=False)
    y_e = nc.declare_dram_parameter("y", [P, 2 * N], BF16, isOutput=True)
    tick_e = None
    if loop_n is not None:
        tick_e = nc.declare_dram_parameter("tick", [1, 8], F32, isOutput=True)

    ext = (x8_e, xb_e, wq8_e, wk8_e, wv8_e, bq_e, bk_e, gam_e, y_e)

    with (
        TileContext(nc) as tc,
        tc.tile_pool(name="consts", bufs=1) as consts,
        tc.tile_pool(name="big", bufs=1) as big,
        tc.tile_pool(name="epool", bufs=12) as epool,
        tc.tile_pool(name="fin", bufs=2) as fin,
        tc.tile_pool(name="ps_big", bufs=2, space="PSUM") as ps_big,
        tc.tile_pool(name="ps_acc", bufs=1, space="PSUM") as ps_acc,
    ):
        pools = (consts, big, epool, fin, ps_big, ps_acc)
        if loop_n is None:
            _emit_body(nc, tc, pools, ext)
        else:
            with tc.For_i(0, loop_n, 1):
                _emit_body(nc, tc, pools, ext)
            t = fin.tile([1, 8], F32, tag="tick")
            nc.vector.memset(t[:], 1.0)
            nc.sync.dma_start(out=tick_e[:], in_=t[:])

    return nc


_NC_CACHE = None


def _get_nc() -> bass.Bass:
    global _NC_CACHE
    if _NC_CACHE is None:
        _NC_CACHE = build_bass()
    return _NC_CACHE


def prep_core_inputs(x, Wq, bq, Wk, bk, Wv, bv, gamma):
    f8 = ml_dtypes.float8_e4m3
    x = np.asarray(x, np.float32).reshape(B, C, N)
    g = float(np.asarray(gamma).reshape(-1)[0])
    bv = np.asarray(bv, np.float32)

    def oq_layout(wT_tiled):  # (C, M) -> (P, 2*M): [p, o*M+m] = wT[o*128+p, m]
        cdim, m = wT_tiled.shape
        return np.ascontiguousarray(
            wT_tiled.reshape(2, P, m).transpose(1, 0, 2).reshape(P, 2 * m)
        )

    wq8 = oq_layout(np.tile(np.asarray(Wq, np.float32).T, (1, 4)) * W_SCALE).astype(f8)
    wk8 = oq_layout(np.tile(np.asarray(Wk, np.float32).T, (1, 4)) * W_SCALE).astype(f8)
    wv8 = oq_layout(np.asarray(Wv, np.float32).T).astype(f8)
    bq_r = (np.tile(np.asarray(bq, np.float32), 4) * W_SCALE).reshape(P, 1)
    bk_r = (np.tile(np.asarray(bk, np.float32), 4) * W_SCALE).reshape(P, 1)
    gam_b = np.full((P, 1), g, np.float32)

    shared = {
        "wq8": wq8, "wk8": wk8, "wv8": wv8,
        "bq_r": np.ascontiguousarray(bq_r), "bk_r": np.ascontiguousarray(bk_r),
        "gam_b": gam_b,
    }
    xg = x + (g * bv)[None, :, None]   # residual + gamma*bv (softmax bias)
    maps = []
    for b in range(B):
        xo = x[b].reshape(2, P, N).transpose(1, 0, 2).reshape(P, 2 * N)
        xgo = xg[b].reshape(2, P, N).transpose(1, 0, 2).reshape(P, 2 * N)
        maps.append({
            "x8": np.ascontiguousarray(xo).astype(f8),
            "xb": np.ascontiguousarray(xgo).astype(ml_dtypes.bfloat16),
            **shared,
        })
    return maps


def kernel(**inputs) -> np.ndarray:
    nc = _get_nc()
    in_maps = prep_core_inputs(**inputs)
    res = run_bass_kernel_spmd(nc, in_maps, list(range(B)))
    y = np.stack([
        res.results[b]["y"].astype(np.float32).reshape(P, 2, N).transpose(1, 0, 2)
        for b in range(B)
    ])  # (B, 2, 128, N)
    return np.ascontiguousarray(y.reshape(B, C, H, W))
